# revision 7
# baseline (speedup 1.0000x reference)
"""Trainium2 Bass kernel v2 for InterpBaselineEncoder (histogram binning).

Reference (per batch b): coarsen 128x128 grid 4x4 -> 32x32 cells; bin U=8192
off-grid points to nearest cell (closed form since grid is regular); scatter-
mean y values + pooled on-grid value per cell; bin T targets, gather cell avg.

Design (per core = one (batch, target-half) pair):
  - cell split r = 4*i + (j>>3) in [0,128), c = j&7 in [0,8): scatter matmuls
    accumulate psum[r=128, (c,y)+cnt = 72] over 64 point tiles of 128 points.
  - all one-hots on DVE in 2x mode via pair-APs: the compare scalar per point
    is materialized twice (pair last dim [1,2]) directly by the binning
    combine ops, and iota constants are read with pair patterns, so every
    operand has a stride-1 last dim (the 2x_1p requirement).
  - binning affine/magic-round steps on the Activation engine; only the two
    tensor+tensor combines run on DVE.
  - on-grid values folded arithmetically: host stages ycON as [r, (c,y), 16]
    so gv16 = reduce(., 16) on Pool; avg = (16*sums + gv16) / (16*cnt + 16).
  - gather: one-hot of target cells with t on partitions, DMA-transposed to
    [cell, t] stationaries; rv = onehot^T @ avg[y,c]; c-onehot mult+reduce.
  - output staged bf16; host converts to f32.
"""
import sys
import numpy as np

for _p in ("/opt/trn_rl_repo", "/opt/pypackages"):
    if _p not in sys.path:
        sys.path.insert(0, _p)

import ml_dtypes  # noqa: E402
from concourse import bass, bacc, mybir, tile  # noqa: E402
from concourse.bass_utils import run_bass_kernel_spmd  # noqa: E402

F32 = mybir.dt.float32
BF16 = mybir.dt.bfloat16
ALU = mybir.AluOpType
ACT_COPY = mybir.ActivationFunctionType.Copy

B, U, T, Y = 4, 8192, 4096, 8
TH = T // 2            # targets per core (2048)
KT = U // 128          # 64 point tiles
NT = TH // 128         # 16 target tiles

_INV = 127.0 / 4.0
_OFF0 = float(np.float32(-(1.5 / 127.0) * _INV))
_MAGIC = 12582912.0  # 1.5*2^23: (z+M)-M rounds to nearest int (ulp=1 domain)

# const bf16 block: iota128 | iota8 | c64cy | ident128
_CB_COLS = 128 + 8 + 64 + 128
# packed f32 coords block [128, 160]: py(64) | ty(16) | px(64) | tx(16)
_INF_COLS = KT + NT + KT + NT


def build_nc(loop_n=0):
    nc = bacc.Bacc("TRN2", target_bir_lowering=False, debug=False)

    cfB = nc.declare_dram_parameter("cfB", [128, _CB_COLS], BF16, isOutput=False)
    inF = nc.declare_dram_parameter("inF", [128, _INF_COLS], F32, isOutput=False)
    inY = nc.declare_dram_parameter("inY", [128, KT * Y], BF16, isOutput=False)
    ycON = nc.declare_dram_parameter("ycON", [128, 1024], BF16, isOutput=False)
    out_d = nc.declare_dram_parameter("out", [TH, Y], BF16, isOutput=True)

    with tile.TileContext(nc) as tc:
        with (
            tc.tile_pool(name="const", bufs=1) as cpool,
            tc.tile_pool(name="work", bufs=1) as wpool,
            tc.tile_pool(name="psS", bufs=1, space="PSUM") as psS,
            tc.tile_pool(name="psT", bufs=2, space="PSUM") as psT,
            tc.tile_pool(name="psR", bufs=2, space="PSUM") as psR,
        ):
            import contextlib
            loop_ctx = tc.For_i(0, loop_n, 1) if loop_n else contextlib.nullcontext()
            with loop_ctx:
                tin = wpool.tile([128, _INF_COLS], F32, tag="tin")
                nc.sync.dma_start(tin[:], inF[:])
                cb = cpool.tile([128, _CB_COLS], BF16, tag="cb")
                nc.sync.dma_start(cb[:], cfB[:])
                ty = wpool.tile([128, KT, Y], BF16, tag="ty")
                nc.scalar.dma_start(ty[:], inY[:].rearrange("p (k y) -> p k y", y=Y))
                tyc = wpool.tile([128, 1024], BF16, tag="tyc")
                nc.sync.dma_start(tyc[:], ycON[:])

                c_iota128 = cb[:, 0:128]          # values 0..127
                c_iota8 = cb[:, 128:136]          # values 0..7
                c_c64 = cb[:, 136:200]            # repeat(arange(8), 8): (c,y)->c
                c_ident = cb[:, 200:328]          # identity (transpose)

                # pair-AP views of consts: [.., pairs, 2] with stride-1 last
                io128h = c_iota128.rearrange("p (a b) -> p a b", b=2)  # [p,64,2]
                io8h = c_iota8.rearrange("p (a b) -> p a b", b=2)      # [p,4,2]
                c64h = c_c64.rearrange("p (a b) -> p a b", b=2)        # [p,32,2]

                # ---- binning: affine/magic steps on ACT, combines on DVE ----
                # layout [py 64 | ty 16 | px 64 | tx 16]; i-part 0:80, j 80:160
                z1 = wpool.tile([128, 160], F32, tag="z1")
                bins = wpool.tile([128, 160], F32, tag="bins")
                nc.vector.tensor_scalar(z1[:], tin[:], _INV, _OFF0,
                                        ALU.mult, ALU.add)
                nc.vector.tensor_scalar(bins[:], z1[:], _MAGIC, _MAGIC,
                                        ALU.add, ALU.subtract)
                iv = bins[:, 0:80]
                jv = bins[:, 80:160]
                # jh2 = round(j/8 - 0.4375); c = j - 8*jh2; r = 4*i + jh2
                t1 = wpool.tile([128, 80], F32, tag="t1")
                jh2 = wpool.tile([128, 80], F32, tag="jh2")
                nc.vector.tensor_scalar(t1[:], jv, 0.125, -0.4375,
                                        ALU.mult, ALU.add)
                nc.vector.tensor_scalar(jh2[:], t1[:], _MAGIC, _MAGIC,
                                        ALU.add, ALU.subtract)
                jh8 = wpool.tile([128, 80], F32, tag="jh8")
                nc.vector.tensor_scalar(jh8[:], jh2[:], 8.0, None, ALU.mult)
                i4 = wpool.tile([128, 80], F32, tag="i4")
                nc.vector.tensor_scalar(i4[:], iv, 4.0, None, ALU.mult)
                # paired bf16 outputs: [128, 80, 2] (value duplicated)
                clp = wpool.tile([128, 80, 2], BF16, tag="clp")
                nc.vector.tensor_tensor(
                    clp[:],
                    jv.unsqueeze(2).broadcast_to((128, 80, 2)),
                    jh8[:].unsqueeze(2).broadcast_to((128, 80, 2)),
                    ALU.subtract)
                rallp = wpool.tile([128, 80, 2], BF16, tag="rallp")
                nc.vector.tensor_tensor(
                    rallp[:],
                    i4[:].unsqueeze(2).broadcast_to((128, 80, 2)),
                    jh2[:].unsqueeze(2).broadcast_to((128, 80, 2)),
                    ALU.add)
                rp_off = rallp[:, 0:KT, :]
                rp_t = rallp[:, KT:80, :]
                cp_off = clp[:, 0:KT, :]
                cp_t = clp[:, KT:80, :]

                # ---- target one-hots ----
                ort = wpool.tile([128, NT, 64, 2], BF16, tag="ort")
                nc.vector.tensor_tensor(
                    ort[:],
                    io128h.unsqueeze(1).broadcast_to((128, NT, 64, 2)),
                    rp_t.unsqueeze(2).broadcast_to((128, NT, 64, 2)),
                    ALU.is_equal,
                )
                zlt = wpool.tile([128, NT, 4, 2], BF16, tag="zlt")
                nc.vector.tensor_tensor(
                    zlt[:],
                    io8h.unsqueeze(1).broadcast_to((128, NT, 4, 2)),
                    cp_t.unsqueeze(2).broadcast_to((128, NT, 4, 2)),
                    ALU.is_equal,
                )

                # ---- scatter pipeline (chunks of 32 tiles) ----
                blr = wpool.tile([128, KT, 32, 2], BF16, tag="blr")
                w1 = wpool.tile([128, KT, 72], BF16, tag="w1")
                ra = wpool.tile([128, KT, 64, 2], BF16, tag="ra")
                ps = psS.tile([128, 72], F32, tag="ps")
                ortv = ort[:].rearrange("p n a b -> p n (a b)")
                rt2T = wpool.tile([128, NT, 128], BF16, tag="rt2T")
                CH = 16
                for c0 in range(0, KT, CH):
                    sl = slice(c0, c0 + CH)
                    # ra[p, k, cell] = (r == cell)
                    nc.vector.tensor_tensor(
                        ra[:, sl, :, :],
                        io128h.unsqueeze(1).broadcast_to((128, CH, 64, 2)),
                        rp_off[:, sl, :].unsqueeze(2)
                        .broadcast_to((128, CH, 64, 2)),
                        ALU.is_equal,
                    )
                    # blr[p, k, (c,y)] = (c == cl): pairs span y-halves
                    nc.vector.tensor_tensor(
                        blr[:, sl, :, :],
                        c64h.unsqueeze(1).broadcast_to((128, CH, 32, 2)),
                        cp_off[:, sl, :].unsqueeze(2)
                        .broadcast_to((128, CH, 32, 2)),
                        ALU.is_equal,
                    )
                    # count columns w1[:, k, 64:72] = blr row (c, y=0)
                    nc.scalar.copy(w1[:, sl, 64:72], blr[:, sl, ::4, 0])
                    # w1[:, k, 0:64] = blr * y  (c outer, y inner; 2x mode)
                    nc.vector.tensor_tensor(
                        w1[:, sl, 0:64].rearrange("p k (c y) -> p k c y", y=Y),
                        blr[:, sl, :, :].rearrange(
                            "p k a b -> p k (a b)").rearrange(
                            "p k (c y) -> p k c y", y=Y),
                        ty[:, sl, :].unsqueeze(2)
                        .broadcast_to((128, CH, 8, Y)),
                        ALU.mult,
                    )
                    for k in range(c0, c0 + CH):
                        nc.tensor.matmul(
                            ps[:],
                            ra[:, k, :, :].rearrange("p a b -> p (a b)"),
                            w1[:, k, :],
                            start=(k == 0), stop=(k == KT - 1))
                    # fill PE's w1-wait gap with a transpose group
                    if c0 // CH < 2:
                        g = (c0 // CH) * 8
                        pt8 = psT.tile([128, 8, 128], BF16, tag="pt8")
                        for j in range(8):
                            nc.tensor.transpose(pt8[:, j, :],
                                                ortv[:, g + j, :], c_ident)
                        nc.scalar.copy(rt2T[:, g:g + 8, :], pt8[:])

                # ---- grid values: gv16[r, (c,y)] = sum of 16 fine ----
                gv16 = wpool.tile([128, 64], BF16, tag="gv16")
                with nc.allow_low_precision(reason="gv16: 16 bf16 terms"):
                    nc.vector.tensor_reduce(
                        gv16[:], tyc[:].rearrange("p (g f) -> p g f", f=16),
                        axis=mybir.AxisListType.X, op=ALU.add)

                # ---- avg[r, y, c] = (16*sums + gv16) / (16*cnt + 16) ----
                s16 = wpool.tile([128, 64], F32, tag="s16")
                nc.vector.tensor_scalar(s16[:], ps[:, 0:64], 16.0, None,
                                        ALU.mult)
                cntw = wpool.tile([128, 8], F32, tag="cntw")
                nc.vector.tensor_scalar(cntw[:], ps[:, 64:72], 16.0, 16.0,
                                        ALU.mult, ALU.add)
                t2 = wpool.tile([128, 64], F32, tag="t2")
                nc.vector.tensor_tensor(t2[:], s16[:], gv16[:], ALU.add)
                rec = wpool.tile([128, 8], F32, tag="rec")
                nc.vector.reciprocal(rec[:], cntw[:])
                avg = wpool.tile([128, Y, 8], BF16, tag="avg")
                nc.vector.tensor_tensor(
                    avg[:],
                    t2[:].rearrange("p (c y) -> p y c", y=Y),
                    rec[:].unsqueeze(1).broadcast_to((128, Y, 8)),
                    ALU.mult,
                )

                # ---- gather (chunks of 4 target tiles) ----
                outsb = wpool.tile([128, NT, Y], BF16, tag="outsb")
                for g in range(0, NT, 4):
                    prv = psR.tile([128, 4, 64], F32, tag="prv")
                    for j in range(4):
                        nc.tensor.matmul(prv[:, j, :], rt2T[:, g + j, :],
                                         avg[:].rearrange("p y c -> p (y c)"),
                                         start=True, stop=True)
                    rv_sb = wpool.tile([128, 4, Y, 8], BF16, tag="rv_sb",
                                       bufs=2)
                    nc.scalar.copy(rv_sb[:], prv[:].rearrange(
                        "p n (y c) -> p n y c", c=8))
                    fm = wpool.tile([128, 4, Y, 8], BF16, tag="fm", bufs=2)
                    nc.vector.tensor_tensor(
                        fm[:], rv_sb[:],
                        zlt[:, g:g + 4, :, :].rearrange("p n a b -> p n (a b)")
                        .unsqueeze(2).broadcast_to((128, 4, Y, 8)),
                        ALU.mult,
                    )
                    with nc.allow_low_precision(
                            reason="one-hot select: one nonzero term"):
                        nc.vector.tensor_reduce(outsb[:, g:g + 4, :], fm[:],
                                                axis=mybir.AxisListType.X,
                                                op=ALU.add)

                nc.sync.dma_start(
                    out_d[:].rearrange("(n p) y -> p n y", p=128), outsb[:])
    nc.compile()
    return nc


def _consts():
    cf = np.zeros((128, _CB_COLS), np.float32)
    cf[:, 0:128] = np.arange(128, dtype=np.float32)[None, :]
    cf[:, 128:136] = np.arange(8, dtype=np.float32)[None, :]
    cf[:, 136:200] = np.repeat(np.arange(8, dtype=np.float32), 8)[None, :]
    cf[:, 200:328] = np.eye(128, dtype=np.float32)
    return {"cfB": cf.astype(ml_dtypes.bfloat16)}


def _stage_core(xc_off, yc_off, yc_on, xt, b, half):
    m = {}
    fin = np.empty((128, _INF_COLS), np.float32)
    sl = slice(half * TH, (half + 1) * TH)
    fin[:, 0:KT] = xc_off[b, :, 0].reshape(KT, 128).T
    fin[:, KT:80] = xt[b, sl, 0].reshape(NT, 128).T
    fin[:, 80:80 + KT] = xc_off[b, :, 1].reshape(KT, 128).T
    fin[:, 80 + KT:160] = xt[b, sl, 1].reshape(NT, 128).T
    m["inF"] = fin
    m["inY"] = np.ascontiguousarray(
        yc_off[b].reshape(KT, 128, Y).transpose(1, 0, 2).reshape(128, KT * Y)
    ).astype(ml_dtypes.bfloat16)
    # ycON: [r=(i,jh), (c, y), fine=(fh,fw)]
    x = yc_on[b].reshape(32, 4, 4, 8, 4, Y)      # i, fh, jh, c, fw, y
    x = x.transpose(0, 2, 3, 5, 1, 4)             # i, jh, c, y, fh, fw
    m["ycON"] = np.ascontiguousarray(x.reshape(128, 1024)).astype(
        ml_dtypes.bfloat16)
    return m


_NC = None


def kernel(xc_off_grid, yc_off_grid, xc_on_grid, yc_on_grid, xt):
    global _NC
    if _NC is None:
        _NC = build_nc()
    nc = _NC
    consts = _consts()

    xc_off_grid = np.ascontiguousarray(xc_off_grid, np.float32)
    yc_off_grid = np.ascontiguousarray(yc_off_grid, np.float32)
    yc_on_grid = np.ascontiguousarray(yc_on_grid, np.float32)
    xt = np.ascontiguousarray(xt, np.float32)

    in_maps = []
    for core in range(8):
        b, half = core // 2, core % 2
        m = dict(consts)
        m.update(_stage_core(xc_off_grid, yc_off_grid, yc_on_grid, xt, b, half))
        in_maps.append(m)

    res = run_bass_kernel_spmd(nc, in_maps, list(range(8)))
    out = np.empty((B, T, Y), np.float32)
    for core in range(8):
        b, half = core // 2, core % 2
        out[b, half * TH:(half + 1) * TH] = \
            res.results[core]["out"].astype(np.float32)
    return out
